# revision 33
# baseline (speedup 1.0000x reference)
"""Trainium2 Bass kernel for multi-head attention (B=2, S=2048, D=1024, H=16, HD=64).

Sharding: tensor-parallel over heads. Each of the 8 cores owns 2 heads
(core c -> heads 2c, 2c+1) and computes:
  - q^T, k^T projections for its heads (layout [head_dim*2, seq])
  - v^T projection + one PE transpose per s-tile into [1|pad63|v(64)] blocks,
    so the attended matmul's stationary operand carries a ones column: the
    attended matmul then emits the softmax denominators on PSUM partition 0
  - scores^T = K @ Q^T per (batch, head) in [key, query] layout -> exp on
    ScalarE (reading PSUM directly, 2-key-tile groups) with the 1/sqrt(64)
    scale fused into the activation
  - attended^T = [1|pad|V]^T @ exp(scores^T), normalized via
    reciprocal_approx_fast + a GpSimd partition broadcast
  - partial output projection out_c = attended_c @ out_w[:, heads_c]^T,
    written out as bf16 partials
Host sums the 8 partial outputs and adds the bias.

Schedule: the ScalarE exp stream (142us of ACT work) and the PE matmul
stream (166us of moving columns) are the two big engine loads; every score
group's exp is fed just-in-time and all exp-independent PE work (the
projections for the *next* batch, the v transposes, and the output
projection tiles of *completed* iterations) is woven between score/attended
groups across the whole run so the PE never idles waiting on exp. Both
heads' attended matmuls run inline one group behind the exps; there is no
deferred cross-iteration tail.
"""

import numpy as np
import ml_dtypes

import concourse.bacc as bacc
import concourse.tile as tile
import concourse.mybir as mybir
from concourse.bass_utils import run_bass_kernel_spmd
from concourse.masks import make_identity

B, S, D = 2, 2048, 1024
H, HD = 16, 64
FEA = H * HD  # 1024
NCORES = 8
BS = B * S  # 4096

DT_TILES = 8      # 1024 contraction dim / 128
JT = 16           # key tiles of 128 per batch
IB = 4            # query blocks of 512 per batch
VW = 256          # v storage width per s-tile: [1|pad63|vA(64) | 1|pad63|vB(64)]

BF16 = mybir.dt.bfloat16
F32 = mybir.dt.float32
AF = mybir.ActivationFunctionType
ALU = mybir.AluOpType

_NC_CACHE = {}


def _emit(tc, xT, wqk, wv, wo, out):
    nc = tc.nc
    with (
        tc.tile_pool(name="consts", bufs=1) as consts,
        tc.tile_pool(name="stp", bufs=3) as stp,
        tc.tile_pool(name="small", bufs=6) as small,
        tc.tile_pool(name="outsb", bufs=3) as outsb,
        tc.tile_pool(name="ps_big", bufs=2, space="PSUM") as ps_big,
        tc.tile_pool(name="ps_att", bufs=2, space="PSUM") as ps_att,
        tc.tile_pool(name="ps_small", bufs=2, space="PSUM") as ps_small,
    ):
        # Coalesced SBUF images: one tile per logical tensor. The host
        # pre-arranges xT/wqk/wv into this exact dt-major layout so the bulk
        # DMAs are fully contiguous (3-8KB per partition line).
        xt = consts.tile([128, DT_TILES * BS], BF16, tag="xt")  # dt-major
        wqks = consts.tile([128, DT_TILES * 256], BF16, tag="wqk")
        wvs = consts.tile([128, DT_TILES * 128], BF16, tag="wv")
        wos = consts.tile([128, D], BF16, tag="wo")
        ones_col = consts.tile([1, 128], F32, tag="ones_col")
        qTs = [consts.tile([128, S], BF16, name=f"qT{b}", tag=f"qT{b}") for b in range(B)]
        kTs = [consts.tile([128, S], BF16, name=f"kT{b}", tag=f"kT{b}") for b in range(B)]
        vsbs = [consts.tile([128, 16 * VW], BF16, name=f"v{b}", tag=f"v{b}") for b in range(B)]
        attTs = [consts.tile([128, S], BF16, name=f"attT{b}", tag=f"attT{b}") for b in range(B)]
        # one shared v^T staging tile: batch 1 overwrites batch 0 after
        # all of batch 0's s-tiles are transposed out (framework-sequenced)
        vT_sh = consts.tile([128, S], BF16, tag="vT")
        ident = consts.tile([128, 128], BF16, tag="ident")

        def xcol(dt, gcol):
            # cb-major layout: global 512-col block cb holds all 8 dt slices
            cb = gcol // 512
            return xt[:, cb * 4096 + dt * 512: cb * 4096 + (dt + 1) * 512]

        def wqk_t(dt):
            return wqks[:, dt * 256:(dt + 1) * 256]

        def wv_t(dt):
            return wvs[:, dt * 128:(dt + 1) * 128]

        # weights + the first x column-block land first so compute can start
        # ~5us in; the rest of x streams in behind as big contiguous per-dt
        # chunks (batch-0 columns first: they're needed ~25us before b1's).
        nc.sync.dma_start(out=wqks, in_=wqk[:, :])
        nc.sync.dma_start(out=xt[:, 0:2048], in_=xT[:, 0:2048])
        nc.sync.dma_start(out=xt[:, 2048:4096], in_=xT[:, 2048:4096])
        nc.sync.dma_start(out=wvs, in_=wv[:, :])
        nc.sync.dma_start(out=wos, in_=wo[:, :])
        for cb in range(1, BS // 512):
            nc.sync.dma_start(
                out=xt[:, cb * 4096:(cb + 1) * 4096],
                in_=xT[:, cb * 4096:(cb + 1) * 4096])
        for b in range(B):
            # only the [1| columns feed real outputs (pad cols land on
            # ignored PSUM rows); a strided memset covers all 16 s-tiles'
            # two ones-columns in one cheap op each.
            ones = vsbs[b].rearrange("p (st two c) -> p st two c", st=16, two=2)
            nc.vector.memset(ones[:, :, :, 0:1], 1.0)
        nc.vector.memset(ones_col, 1.0)
        make_identity(nc, ident)

        def _emit_proj_nb(b, nb, half, dst):
            # one 512-col block of a q^T / k^T projection
            scol = nb * 512
            ps = ps_small.tile([128, 512], F32, name="pss", tag="ps_small")
            for dt in range(DT_TILES):
                nc.tensor.matmul(
                    ps,
                    lhsT=wqk_t(dt)[:, half * 128:(half + 1) * 128],
                    rhs=xcol(dt, b * S + scol),
                    start=(dt == 0),
                    stop=(dt == DT_TILES - 1),
                )
            nc.vector.tensor_copy(out=dst[:, scol:scol + 512], in_=ps)

        def emit_q_nb(b, nb):
            _emit_proj_nb(b, nb, 0, qTs[b])

        def emit_k_nb(b, nb):
            _emit_proj_nb(b, nb, 1, kTs[b])

        def emit_vT_nb(b, nb):
            scol = nb * 512
            ps = ps_small.tile([128, 512], F32, name="pss", tag="ps_small")
            for dt in range(DT_TILES):
                nc.tensor.matmul(
                    ps,
                    lhsT=wv_t(dt),
                    rhs=xcol(dt, b * S + scol),
                    start=(dt == 0),
                    stop=(dt == DT_TILES - 1),
                )
            nc.vector.tensor_copy(out=vT_sh[:, scol:scol + 512], in_=ps)

        def emit_v(b, st):
            # One PE transpose turns v^T's [f=128, s-tile] block into natural
            # [s, f] order, then a strided DVE copy lands it as
            # [1|pad63|vA(64) | 1|pad63|vB(64)].
            ps = ps_small.tile([128, 128], BF16, name="pss", tag="ps_small")
            nc.tensor.transpose(
                ps, vT_sh[:, st * 128:(st + 1) * 128], ident
            )
            vsrc = ps.rearrange("p (two c) -> p two c", two=2)
            vdst = vsbs[b][:, st * VW: st * VW + VW].rearrange(
                "p (two c) -> p two c", two=2
            )[:, :, 64:128]
            nc.vector.tensor_copy(out=vdst, in_=vsrc)

        def emit_outproj_st(b, ib, st, split=False):
            # output rows [b*S + (b*16+ib*4+st_rel)*128, +128): one stationary
            # (the attT block), both 512-wide halves of wo, one bf16 store.
            # split=True contracts per head (K=64 + K=64 accumulate) so the
            # head-A half can run before head B's normalize lands (tail).
            st_abs = b * 16 + ib * 4 + st
            acol = (ib * 4 + st) * 128
            osb = outsb.tile([128, D], BF16, name="osb", tag="osb")
            for db in range(2):
                ps = ps_small.tile([128, 512], F32, name="pss", tag="ps_small")
                if split:
                    for h in range(2):
                        nc.tensor.matmul(
                            ps,
                            lhsT=attTs[b][h * 64:(h + 1) * 64, acol:acol + 128],
                            rhs=wos[h * 64:(h + 1) * 64, db * 512:(db + 1) * 512],
                            start=(h == 0),
                            stop=(h == 1),
                        )
                else:
                    nc.tensor.matmul(
                        ps,
                        lhsT=attTs[b][:, acol:acol + 128],
                        rhs=wos[:, db * 512:(db + 1) * 512],
                        start=True,
                        stop=True,
                    )
                nc.vector.tensor_copy(out=osb[:, db * 512:(db + 1) * 512], in_=ps)
            nc.sync.dma_start(
                out=out[st_abs * 128:(st_abs + 1) * 128, :], in_=osb)

        def normalize_phase1(st):
            # Both heads' chains interleaved so they pipeline across DVE/PE.
            # Drain PSUM immediately (one copy per head) so the banks free
            # fast. Row 0 of the drain = softmax denominators, rows 64:128 =
            # attended^T.
            for h, ps_ in ((0, st["attA"]), (1, st["attB"])):
                araw = small.tile([128, 512], F32, name="araw", tag="araw")
                nc.vector.tensor_copy(out=araw, in_=ps_[0:128, :])
                st[f"araw{h}"] = araw
            for h in range(2):
                rrow = small.tile([1, 512], F32, name="rrow", tag="rrow")
                nc.vector.reciprocal_approx_fast(
                    out=rrow, in_=st[f"araw{h}"][0:1, :])
                st[f"rrow{h}"] = rrow

        def normalize_phase2(st, pe_bcast=False):
            # Broadcast the per-query 1/denominator across partitions, then
            # scale; head A lands on attT partitions 0:64 via the tensor
            # op's partition shift, head B multiplies in place at partitions
            # 64:128. At the kernel tail the broadcast runs as a fp32
            # ones-column matmul on the (idle) PE instead of the 1us GpSimd
            # partition_broadcast, shortening the final normalize chain.
            b, icol = st["b"], st["ib"] * 512
            rbs = [None, None]
            for h in range(2):
                if pe_bcast:
                    rbs[h] = ps_small.tile([128, 512], F32, name="rbp", tag="ps_small")
                    nc.tensor.matmul(
                        rbs[h], lhsT=ones_col, rhs=st[f"rrow{h}"],
                        start=True, stop=True)
                else:
                    rbs[h] = small.tile([128, 512], F32, name="rb", tag="rb")
                    nc.gpsimd.partition_broadcast(rbs[h], st[f"rrow{h}"])
            for h in range(2):
                nc.vector.tensor_tensor(
                    out=attTs[b][h * 64:(h + 1) * 64, icol:icol + 512],
                    in0=st[f"araw{h}"][64:128, :],
                    in1=rbs[h][64:128, :],
                    op=ALU.mult,
                )

        # ---- filler schedule -------------------------------------------------
        # (iteration index 0..7, group 0..7) -> exp-independent PE work to
        # weave in. Projections must precede the scores/attended that read
        # them; outproj units follow the iteration whose normalize produced
        # their attT block. x chunks arrive ~1 per 2.5us, cb c covers
        # b=c//4, s-cols [c%4 *512, +512).
        def F(kind, *a):
            return (kind, a)

        WEAVE = {
            # iter 0 = (b0, ib0): own k blocks ahead of the score groups that
            # read them (scores g read k-tiles 2g,2g+1; k block nb covers
            # tiles 4nb..4nb+3), v transposes ahead of the attended stream
            # (attended at group g reads v s-tiles 2g-2, 2g-1).
            (0, 1): (F("vT", 0, 0), F("k", 0, 1), F("v", 0, 0), F("v", 0, 1)),
            (0, 2): (F("k", 0, 2), F("vT", 0, 1), F("v", 0, 2), F("v", 0, 3)),
            (0, 3): (F("v", 0, 4), F("v", 0, 5)),
            (0, 4): (F("k", 0, 3), F("vT", 0, 2), F("v", 0, 6), F("v", 0, 7)),
            (0, 5): (F("v", 0, 8), F("v", 0, 9)),
            (0, 6): (F("vT", 0, 3), F("v", 0, 10), F("v", 0, 11)),
            (0, 7): (F("v", 0, 12), F("v", 0, 13), F("v", 0, 14), F("v", 0, 15), F("q", 0, 1)),
            # iter 1 = (b0, ib1): batch-1 q/k projections (x chunks 4-7 have
            # landed by ~25us), plus b0's remaining q blocks just before the
            # iterations that read them.
            (1, 0): (F("k", 1, 0),),
            (1, 1): (F("q", 1, 0),),
            (1, 2): (F("k", 1, 1),),
            (1, 3): (F("q", 1, 1),),
            (1, 4): (F("k", 1, 2),),
            (1, 5): (F("q", 1, 2),),
            (1, 6): (F("k", 1, 3), F("q", 0, 2)),
            (1, 7): (F("q", 1, 3),),
            # iter 2 = (b0, ib2): batch-1 v pipeline + outproj of iter 1
            (2, 0): (F("vT", 1, 0),),
            (2, 1): (F("vT", 1, 1),),
            (2, 2): (F("vT", 1, 2), F("v", 1, 0), F("v", 1, 1)),
            (2, 3): (F("vT", 1, 3), F("v", 1, 2), F("v", 1, 3)),
            (2, 4): (F("v", 1, 4), F("v", 1, 5)),
            (2, 5): (F("v", 1, 6), F("v", 1, 7)),
            (2, 6): (F("v", 1, 8), F("v", 1, 9), F("q", 0, 3)),
            (2, 7): (F("v", 1, 10), F("v", 1, 11)),
            # iter 3 = (b0, ib3)
            (3, 0): (F("v", 1, 12), F("v", 1, 13)),
            (3, 1): (F("v", 1, 14), F("v", 1, 15)),
            # all outproj units live in the ACT-paced batch-1 phase where
            # the PE otherwise idles; the PE-bound early phase keeps only
            # the mandatory projection fillers
            (4, 1): (F("op", 0, 0, 0),),
            (4, 2): (F("op", 0, 0, 1),),
            (4, 3): (F("op", 0, 0, 2),),
            (4, 4): (F("op", 0, 0, 3),),
            (4, 5): (F("op", 0, 1, 0),),
            (4, 6): (F("op", 0, 1, 1),),
            (4, 7): (F("op", 0, 1, 2),),
            (4, 8): (F("op", 0, 1, 3),),
            (5, 1): (F("op", 0, 2, 0),),
            (5, 2): (F("op", 0, 2, 1),),
            (5, 3): (F("op", 0, 2, 2),),
            (5, 4): (F("op", 0, 2, 3),),
            (5, 5): (F("op", 0, 3, 0),),
            (5, 6): (F("op", 0, 3, 1),),
            (5, 7): (F("op", 0, 3, 2),),
            (5, 8): (F("op", 0, 3, 3),),
            (6, 1): (F("op", 1, 0, 0),),
            (6, 2): (F("op", 1, 0, 1),),
            (6, 3): (F("op", 1, 0, 2),),
            (6, 4): (F("op", 1, 0, 3),),
            (6, 5): (F("op", 1, 1, 0),),
            (6, 6): (F("op", 1, 1, 1),),
            (6, 7): (F("op", 1, 1, 2),),
            (6, 8): (F("op", 1, 1, 3),),
            (7, 1): (F("op", 1, 2, 0),),
            (7, 2): (F("op", 1, 2, 1),),
            (7, 3): (F("op", 1, 2, 2),),
            (7, 4): (F("op", 1, 2, 3),),
        }

        def run_filler(kind, a):
            if kind == "q":
                emit_q_nb(*a)
            elif kind == "k":
                emit_k_nb(*a)
            elif kind == "vT":
                emit_vT_nb(*a)
            elif kind == "v":
                emit_v(*a)
            elif kind == "op":
                emit_outproj_st(*a)

        # the previous iteration's last attended pair + normalize run at the
        # START of the next iteration, right after its first score group, so
        # the ScalarE stream never gaps at iteration boundaries.
        pend_fin = {}

        def finish_phase1():
            if not pend_fin:
                return
            for ps_, st_, off in (
                (pend_fin["attA"], pend_fin["stA"], 0),
                (pend_fin["attB"], pend_fin["stB"], 128),
            ):
                jts = pend_fin["jts"]
                for jt in jts:
                    nc.tensor.matmul(
                        ps_[0:128, :],
                        lhsT=vsbs[pend_fin["b"]][:, jt * VW + off: jt * VW + off + 128],
                        rhs=st_[:, jt * 512:(jt + 1) * 512],
                        start=False,
                        stop=(jt == JT - 1),
                    )
            normalize_phase1(pend_fin)

        def finish_phase2(pe_bcast=False):
            if not pend_fin:
                return
            normalize_phase2(pend_fin, pe_bcast=pe_bcast)
            pend_fin.clear()

        # batch-0 iterations use 8 uniform 2-jt score groups. Batch-1
        # iterations (the ACT-paced phase) use 1-jt first/last groups whose
        # score PSUM comes from ps_small: the boundary scores then don't
        # wait on the previous group's exp to free a ps_big slot, so the
        # ScalarE exp stream crosses iteration boundaries without a gap.
        GROUPS_B0 = [(2 * g, 2) for g in range(8)]
        GROUPS_B1 = [(0, 1)] + [(1 + 2 * g, 2) for g in range(7)] + [(15, 1)]

        def emit_attention_ib(it, b, ib):
            icol = ib * 512
            groups = GROUPS_B1 if b == 1 else GROUPS_B0
            stA = stp.tile([128, JT * 512], BF16, name="stA", tag="st")
            stB = stp.tile([128, JT * 512], BF16, name="stB", tag="st")
            attA_ps = ps_att.tile([128, 512], F32, name="attA", tag="att")
            attB_ps = ps_att.tile([128, 512], F32, name="attB", tag="att")
            prev_jts = []
            for g, (g0, gw) in enumerate(groups):
                if g > 0:
                    # fillers first: they run while the score matmuls below
                    # wait for their PSUM slot (freed by exp(g-1)).
                    for kind, a in WEAVE.get((it, g), ()):
                        run_filler(kind, a)
                if gw == 2:
                    scA = ps_big.tile([128, 1024], F32, name="scA", tag="sc")
                    scB = ps_big.tile([128, 1024], F32, name="scB", tag="sc")
                else:
                    scA = ps_small.tile([128, 512], F32, name="scAs", tag="ps_small")
                    scB = ps_small.tile([128, 512], F32, name="scBs", tag="ps_small")
                for idx in range(gw):
                    jt = g0 + idx
                    for hsl, sc in ((slice(0, 64), scA), (slice(64, 128), scB)):
                        nc.tensor.matmul(
                            sc[:, idx * 512:(idx + 1) * 512],
                            lhsT=kTs[b][hsl, jt * 128:(jt + 1) * 128],
                            rhs=qTs[b][hsl, icol:icol + 512],
                            start=True,
                            stop=True,
                        )
                for st_exp, sc in ((stA, scA), (stB, scB)):
                    nc.scalar.activation(
                        out=st_exp[:, g0 * 512:(g0 + gw) * 512],
                        in_=sc,
                        func=AF.Exp,
                        scale=0.125,
                    )
                if g == 0:
                    finish_phase1()
                    for kind, a in WEAVE.get((it, 0), ()):
                        run_filler(kind, a)
                    finish_phase2()
                # attended for both heads consumes the PREVIOUS group's exps
                # (one group of slack so the PE never waits on ScalarE)
                for jt in prev_jts:
                    for ps_, st_, off in ((attA_ps, stA, 0), (attB_ps, stB, 128)):
                        nc.tensor.matmul(
                            ps_[0:128, :],
                            lhsT=vsbs[b][:, jt * VW + off: jt * VW + off + 128],
                            rhs=st_[:, jt * 512:(jt + 1) * 512],
                            start=(jt == 0),
                            stop=False,
                        )
                prev_jts = [g0 + idx for idx in range(gw)]
            pend_fin.update(b=b, ib=ib, stA=stA, stB=stB,
                            attA=attA_ps, attB=attB_ps, jts=prev_jts)

        # prologue: the first iteration's q/k blocks (everything else is
        # woven into the attention stream)
        emit_k_nb(0, 0)
        emit_q_nb(0, 0)
        it = 0
        for b in range(B):
            for ib in range(IB):
                emit_attention_ib(it, b, ib)
                it += 1
        # tail: finish + output projection of the last iteration
        finish_phase1()
        finish_phase2(pe_bcast=True)
        for st in range(4):
            emit_outproj_st(1, 3, st)


def build_nc():
    if "nc" in _NC_CACHE:
        return _NC_CACHE["nc"]
    nc = bacc.Bacc("TRN2", debug=False, num_devices=NCORES)
    # inputs arrive pre-arranged in the dt-major SBUF layout
    xT = nc.dram_tensor("xT", [128, DT_TILES * BS], BF16, kind="ExternalInput").ap()
    wqk = nc.dram_tensor("wqk", [128, DT_TILES * 256], BF16, kind="ExternalInput").ap()
    wv = nc.dram_tensor("wv", [128, DT_TILES * 128], BF16, kind="ExternalInput").ap()
    wo = nc.dram_tensor("wo", [128, D], BF16, kind="ExternalInput").ap()
    out = nc.dram_tensor("out", [BS, D], BF16, kind="ExternalOutput").ap()
    with tile.TileContext(nc) as tc:
        _emit(tc, xT, wqk, wv, wo, out)
    nc.compile()
    _NC_CACHE["nc"] = nc
    return nc


def _dt_major(a):
    """[D, C] -> [128, 8*C] with [p, dt*C + c] = a[dt*128 + p, c]."""
    d, c = a.shape
    return np.ascontiguousarray(
        a.reshape(DT_TILES, 128, c).transpose(1, 0, 2).reshape(128, DT_TILES * c)
    )


def _cb_major(a):
    """[D, BS] -> [128, BS*8] with [p, cb*4096 + dt*512 + c] =
    a[dt*128 + p, cb*512 + c]: every 512-column block of the sequence is a
    fully contiguous [128, 4096] slab (8KB DMA lines)."""
    arr = a.reshape(DT_TILES, 128, BS // 512, 512)
    return np.ascontiguousarray(
        arr.transpose(1, 2, 0, 3).reshape(128, DT_TILES * BS))


def make_in_maps(x, qkv_w):
    """Host-side shard + transpose + cast into the kernel's dt-major SBUF
    layout. Returns per-core input dicts (without wo/out, added by caller)."""
    bf = ml_dtypes.bfloat16
    xT = _cb_major(x.reshape(BS, D).T).astype(bf)            # [128, 8*BS]
    maps = []
    for c in range(NCORES):
        wA = qkv_w[c * 384: c * 384 + 192]
        wB = qkv_w[c * 384 + 192: c * 384 + 384]
        wq = np.concatenate([wA[0:64], wB[0:64]], 0)        # [128, D]
        wk = np.concatenate([wA[64:128], wB[64:128]], 0)    # [128, D]
        wv_ = np.concatenate([wA[128:192], wB[128:192]], 0)  # [128, D]
        wqk_c = _dt_major(np.concatenate([wq, wk], 0).T).astype(bf)  # [128, 8*256]
        wv_c = _dt_major(wv_.T).astype(bf)                   # [128, 8*128]
        maps.append({"xT": xT, "wqk": wqk_c, "wv": wv_c})
    return maps


def kernel(x, qkv_w, out_w, out_b, _run_kwargs=None):
    x = np.asarray(x, dtype=np.float32)
    qkv_w = np.asarray(qkv_w, dtype=np.float32)
    out_w = np.asarray(out_w, dtype=np.float32)
    out_b = np.asarray(out_b, dtype=np.float32)
    bf = ml_dtypes.bfloat16

    nc = build_nc()
    in_maps = make_in_maps(x, qkv_w)
    for c in range(NCORES):
        wo_c = np.ascontiguousarray(
            out_w[:, c * 128:(c + 1) * 128].T).astype(bf)    # [128, D]
        in_maps[c]["wo"] = wo_c

    res = run_bass_kernel_spmd(
        nc, in_maps, list(range(NCORES)), **(_run_kwargs or {})
    )
    total = np.zeros((BS, D), np.float32)
    for c in range(NCORES):
        total += np.asarray(res.results[c]["out"]).astype(np.float32)
    total += out_b[None, :]
    out = total.reshape(B, S, D)
    if _run_kwargs:
        kernel.last_result = res
    return out
